# revision 28
# baseline (speedup 1.0000x reference)
"""Trainium2 Bass kernel for Llama GQA attention (B=2, S=2048, H=4096,
32 Q heads / 8 KV heads, head_dim 128, RoPE, causal).

Sharding: tensor-parallel by head across 8 cores. Core c owns Q heads
[4c..4c+3] and KV head c. Each core computes its Q/K/V projections,
RoPE, causal attention, and a partial output projection over its 512
attention features; the host sums the 8 partial outputs (bf16).

v2 layout decisions (vs the DRAM-scratch baseline):
  - q/k/v and attn live in SBUF end-to-end; no DRAM round trip, no
    phase-2 reload stall (which also caused a HAM re-throttle).
  - V is transposed to [tok, d] at eviction time with PE transposes.
  - Weight loads ride the ACT HWDGE ring as a few big 3D DMAs while
    the x-token stream owns the SP ring; first matmul starts ~5us in.
  - A short burst of dummy matmuls at t=0 warms the PE clock (HAM).
  - Phase 2 is qb-major: attention for all 4 heads of a 512-token
    q-block, then that block's 32 output-projection tiles, so o-proj
    DMA spreads across the whole phase instead of piling into a tail.
  - Output partials are written bf16 (halves output DMA bytes).
"""
import math
import numpy as np
import ml_dtypes

import concourse.bacc as bacc
import concourse.tile as tile
from concourse import mybir
from concourse.bass_utils import run_bass_kernel_spmd

F32 = mybir.dt.float32
BF16 = mybir.dt.bfloat16
NPBF = ml_dtypes.bfloat16

P = 128
B, S, H = 2, 2048, 4096
T = B * S
DK = 128
NHL = 4                      # Q heads per core
FL = NHL * DK                # 512 q features per core
TB = 512                     # token block in phase 1
NTB = T // TB
NA = H // P                  # 32 contraction slices
NAG = 4                      # a-slices per x DMA
QBS = 512                    # q block in phase 2
NQB = S // QBS
NKT = S // P
SCALE = 1.0 / math.sqrt(DK)
NOB = H // 512               # 8 output-column blocks

_NC_CACHE = {}


def build():
    nc = bacc.Bacc(None, target_bir_lowering=False)

    # weights and x arrive pre-transposed to partition-major SBUF layout so
    # each load is one DMA with large contiguous per-partition descriptors
    # (small-descriptor streams starve under packet-level SDMA round-robin)
    xt = nc.dram_tensor("xt", [NTB * P, NA * TB], BF16, kind="ExternalInput")
    wqt = nc.dram_tensor("wqt", [P, NA * FL], BF16, kind="ExternalInput")
    wkt = nc.dram_tensor("wkt", [P, NA * DK], BF16, kind="ExternalInput")
    wvt = nc.dram_tensor("wvt", [P, NA * DK], BF16, kind="ExternalInput")
    wot = nc.dram_tensor("wot", [P, NHL * H], BF16, kind="ExternalInput")
    cost = nc.dram_tensor("cost", [P, S], BF16, kind="ExternalInput")
    sints = nc.dram_tensor("sints", [P, S], BF16, kind="ExternalInput")
    trimask = nc.dram_tensor("trimask", [P, P], BF16, kind="ExternalInput")
    identb = nc.dram_tensor("identb", [P, P], BF16, kind="ExternalInput")
    onesc = nc.dram_tensor("onesc", [P, 1], BF16, kind="ExternalInput")
    out = nc.dram_tensor("out", [T, H], BF16, kind="ExternalOutput")

    EXP = mybir.ActivationFunctionType.Exp

    with nc.allow_low_precision(reason="attention compute dtypes are "
                                       "deliberately reduced"), \
         tile.TileContext(nc) as tc:
        with tc.tile_pool(name="const", bufs=1) as cp, \
             tc.tile_pool(name="resid", bufs=1) as rsp, \
             tc.tile_pool(name="wo", bufs=1) as wop:
            # ---- persistent SBUF tensors ----
            cos_sb = cp.tile([P, S], BF16)
            sin_sb = cp.tile([P, S], BF16)
            tri_sb = cp.tile([P, P], BF16)
            id_sb = cp.tile([P, P], BF16)
            oc_sb = cp.tile([P, 1], BF16)
            q_sb = [[rsp.tile([P, S], BF16, name=f"q{b}_{j}")
                     for j in range(NHL)] for b in range(B)]
            k_sb = [rsp.tile([P, S], BF16, name=f"k{b}") for b in range(B)]
            vtk_sb = [rsp.tile([P, NKT, P], BF16, name=f"vt{b}")
                      for b in range(B)]
            attn_sb = [[rsp.tile([P, S], BF16, name=f"attn{b}_{h}")
                        for h in range(NHL)] for b in range(B)]
            wo_sb = wop.tile([P, NHL, H], BF16)

            # ---- bulk loads on the ACT ring, in need-order: the first
            # token block needs wk/wv/wq[a] progressively, then the
            # eviction/RoPE chain needs id + cos/sin. wo rides the gpsimd
            # ring mid-phase-1 so it doesn't contend at startup.
            wk_sb = cp.tile([P, NA, DK], BF16)
            wv_sb = cp.tile([P, NA, DK], BF16)
            wq_sb = cp.tile([P, NA, FL], BF16)
            wk_view = wkt[:, :].rearrange("p (a f) -> p a f", a=NA)
            wv_view = wvt[:, :].rearrange("p (a f) -> p a f", a=NA)
            wq_view = wqt[:, :].rearrange("p (a f) -> p a f", a=NA)
            nc.scalar.dma_start(out=wk_sb[:, :8, :], in_=wk_view[:, :8, :])
            nc.scalar.dma_start(out=wv_sb[:, :8, :], in_=wv_view[:, :8, :])
            nc.scalar.dma_start(out=wq_sb[:, :8, :], in_=wq_view[:, :8, :])
            nc.scalar.dma_start(out=wk_sb[:, 8:, :], in_=wk_view[:, 8:, :])
            nc.scalar.dma_start(out=wv_sb[:, 8:, :], in_=wv_view[:, 8:, :])
            nc.scalar.dma_start(out=wq_sb[:, 8:16, :], in_=wq_view[:, 8:16, :])
            nc.scalar.dma_start(out=id_sb, in_=identb[:, :])
            nc.scalar.dma_start(out=oc_sb, in_=onesc[:, :])
            nc.scalar.dma_start(out=cos_sb, in_=cost[:, :])
            nc.scalar.dma_start(out=sin_sb, in_=sints[:, :])
            for g in range(2, 4):
                nc.scalar.dma_start(out=wq_sb[:, g * 8:(g + 1) * 8, :],
                                    in_=wq_view[:, g * 8:(g + 1) * 8, :])
            nc.scalar.dma_start(out=tri_sb, in_=trimask[:, :])

            # ---------------- Phase 1: QKV projection + RoPE --------------
            with tc.tile_pool(name="xp", bufs=2) as xp, \
                 tc.tile_pool(name="rp", bufs=1) as rp, \
                 tc.tile_pool(name="ps1", bufs=1, space="PSUM") as ps1:

                def rope(src, dst, s0, uid):
                    # dst = src*cos + swap_halves(src)*sints  (all bf16)
                    sw = rp.tile([P, TB], BF16, name=f"sw_{uid}",
                                 tag="sw", bufs=6)
                    nc.gpsimd.dma_start(out=sw[0:64, :], in_=src[64:128, :])
                    nc.gpsimd.dma_start(out=sw[64:128, :], in_=src[0:64, :])
                    nc.vector.tensor_mul(src, src, cos_sb[:, s0:s0 + TB])
                    nc.vector.tensor_mul(sw, sw, sin_sb[:, s0:s0 + TB])
                    nc.vector.tensor_add(dst, src, sw)

                for tb in range(NTB):
                    if tb == 4:
                        # 4MB wo load on the gpsimd ring; the tiny copy
                        # into its corner pins it behind tb3 (the Tile
                        # scheduler would otherwise hoist it to t=0 where
                        # its bulk starves the startup-critical loads)
                        nc.gpsimd.tensor_copy(wo_sb[0:1, 0, 0:2],
                                              k_sb[0][0:1, 1536:1538])
                        nc.gpsimd.dma_start(
                            out=wo_sb,
                            in_=wot[:, :].rearrange("p (j o) -> p j o",
                                                    j=NHL))
                    bi = (tb * TB) // S
                    s0 = (tb * TB) % S
                    psq = [ps1.tile([P, TB], F32, name=f"psq{j}_{tb}",
                                    tag=f"psq{j}") for j in range(NHL)]
                    psk = ps1.tile([P, TB], F32, name=f"psk_{tb}", tag="psk")
                    psv = ps1.tile([P, TB], F32, name=f"psv_{tb}", tag="psv")
                    chunks = ([(0, 4), (4, 4), (8, 8), (16, 8), (24, 8)]
                              if tb == 0 else
                              [(0, 8), (8, 8), (16, 8), (24, 8)])
                    for a0, nag in chunks:
                        x_t = xp.tile([P, 8, TB], BF16,
                                      name=f"x_{tb}_{a0}", tag="xt")
                        nc.sync.dma_start(
                            out=x_t[:, :nag, :],
                            in_=xt[tb * P:(tb + 1) * P,
                                   a0 * TB:(a0 + nag) * TB].rearrange(
                                       "p (a t) -> p a t", a=nag))
                        for ai in range(nag):
                            a = a0 + ai
                            st, sp = (a == 0), (a == NA - 1)
                            nc.tensor.matmul(psk, wk_sb[:, a, :],
                                             x_t[:, ai, :], start=st, stop=sp)
                            nc.tensor.matmul(psv, wv_sb[:, a, :],
                                             x_t[:, ai, :], start=st, stop=sp)
                            for j in range(NHL):
                                nc.tensor.matmul(
                                    psq[j],
                                    wq_sb[:, a, j * DK:(j + 1) * DK],
                                    x_t[:, ai, :], start=st, stop=sp)

                    # evictions: K first (next tb's first matmul is psk)
                    kc = rp.tile([P, TB], BF16, name=f"kc_{tb}", tag="kc",
                                 bufs=2)
                    nc.scalar.copy(kc, psk)
                    rope(kc, k_sb[bi][:, s0:s0 + TB], s0, f"k{tb}")
                    vb = rp.tile([P, TB], BF16, name=f"vb_{tb}", tag="vb",
                                 bufs=2)
                    nc.vector.tensor_copy(vb, psv)
                    vt_ps = ps1.tile([P, TB], BF16, name=f"vt_{tb}", tag="vt")
                    for m in range(4):
                        nc.tensor.transpose(vt_ps[:, m * P:(m + 1) * P],
                                            vb[:, m * P:(m + 1) * P], id_sb)
                    kt0 = (s0 // P)
                    nc.scalar.copy(vtk_sb[bi][:, kt0:kt0 + 4, :], vt_ps)
                    for j in range(NHL):
                        qc = rp.tile([P, TB], BF16, name=f"qc_{tb}_{j}",
                                     tag="qc", bufs=6)
                        if j % 2 == 0:
                            nc.vector.tensor_copy(qc, psq[j])
                        else:
                            nc.scalar.copy(qc, psq[j])
                        rope(qc, q_sb[bi][j][:, s0:s0 + TB], s0, f"q{tb}_{j}")

            # ------------- Phase 2: attention + output projection ---------
            # qb runs 3..0 so the phase starts with the deepest kt
            # pipeline (absorbs the tb7 eviction/RoPE epilogue latency).
            # Full (non-diagonal) exp tiles are pre-reduced 4:1 on the DVE
            # so the softmax-denominator matmuls stream 4x fewer columns.
            with tc.tile_pool(name="p2", bufs=1) as p2, \
                 tc.tile_pool(name="p2e", bufs=5) as p2e, \
                 tc.tile_pool(name="p3o", bufs=4) as p3o, \
                 tc.tile_pool(name="ps2s", bufs=3, space="PSUM") as ps2s, \
                 tc.tile_pool(name="ps2u", bufs=3, space="PSUM") as ps2u:
                ocnt = [0]

                def emit_otile(b, ti, ob2):
                    # one [128, 1024] tile covering output blocks 2*ob2,
                    # 2*ob2+1; a single paired DMA on the SP ring
                    o_sb = p3o.tile([P, 1024], BF16, name=f"os_{ocnt[0]}",
                                    tag="os")
                    for half in range(2):
                        ob = 2 * ob2 + half
                        o_ps = ps2u.tile([P, 512], F32,
                                         name=f"o_{ocnt[0]}_{half}",
                                         tag="u", bufs=3)
                        for j in range(NHL):
                            nc.tensor.matmul(
                                o_ps, attn_sb[b][j][:, ti * P:(ti + 1) * P],
                                wo_sb[:, j, ob * 512:(ob + 1) * 512],
                                start=(j == 0), stop=(j == NHL - 1))
                        dst = o_sb[:, half * 512:(half + 1) * 512]
                        if (ocnt[0] + half) % 2 == 0:
                            nc.vector.tensor_copy(dst, o_ps)
                        else:
                            nc.scalar.copy(dst, o_ps)
                    r0 = b * S + ti * P
                    eng = nc.sync if ocnt[0] % 2 == 0 else nc.scalar
                    eng.dma_start(
                        out=out[r0:r0 + P, ob2 * 1024:(ob2 + 1) * 1024],
                        in_=o_sb)
                    ocnt[0] += 1

                # o-proj tiles are produced per finished q-block and
                # consumed one-per-kt inside later attention blocks, so
                # PE fills the slack while ACT streams the exps
                pend = []
                for b in range(B):
                    for qb in range(NQB - 1, -1, -1):
                        nkt = 4 * qb + 4
                        for h in range(NHL):
                            u_ps = ps2u.tile([P, QBS], F32,
                                             name=f"u_{b}_{h}_{qb}", tag="u",
                                             bufs=3)
                            d_ps = ps2u.tile([1, QBS], F32,
                                             name=f"d_{b}_{h}_{qb}", tag="d",
                                             bufs=2)
                            dflag = [True]

                            def emit_d(src, lo, sp, d_ps=d_ps):
                                nc.tensor.matmul(d_ps[:, lo:], oc_sb,
                                                 src[:, lo:],
                                                 start=dflag[0], stop=sp,
                                                 skip_group_check=True)
                                dflag[0] = False

                            def emit_av(kt, e_sb, lo, u_ps=u_ps, nkt=nkt,
                                        b=b):
                                st, sp = (kt == 0), (kt == nkt - 1)
                                nc.tensor.matmul(u_ps[:, lo:],
                                                 vtk_sb[b][:, kt, :],
                                                 e_sb[:, lo:],
                                                 start=st, stop=sp,
                                                 skip_group_check=True)

                            av_fifo = []
                            diag_e = []   # diagonal e tiles (post-mask)
                            fulls = []    # full e tiles awaiting pair add
                            pairs = []    # pair sums awaiting quad add
                            quads = []    # (ready_kt, quad tile) for d-mm
                            for kt in range(nkt):
                                s_ps = ps2s.tile(
                                    [P, QBS], F32,
                                    name=f"s_{b}_{h}_{qb}_{kt}", tag="s")
                                m = kt - 4 * qb
                                lo = m * P if m > 0 else 0
                                nc.tensor.matmul(
                                    s_ps[:, lo:],
                                    k_sb[b][:, kt * P:(kt + 1) * P],
                                    q_sb[b][h][:, qb * QBS + lo:
                                               (qb + 1) * QBS],
                                    start=True, stop=True)
                                e_sb = p2e.tile(
                                    [P, QBS], BF16,
                                    name=f"e_{b}_{h}_{qb}_{kt}", tag="e")
                                nc.scalar.activation(e_sb[:, lo:],
                                                     s_ps[:, lo:], EXP,
                                                     scale=SCALE)
                                if m >= 0:
                                    nc.vector.tensor_mul(
                                        e_sb[:, m * P:(m + 1) * P],
                                        e_sb[:, m * P:(m + 1) * P],
                                        tri_sb)
                                    diag_e.append(e_sb)
                                else:
                                    # 4:1 DVE pre-reduction of full tiles
                                    # for the denominator matmul
                                    fulls.append(e_sb)
                                    if len(fulls) == 2:
                                        es = p2.tile(
                                            [P, QBS], BF16,
                                            name=f"ep_{b}_{h}_{qb}_{kt}",
                                            tag="ep", bufs=3)
                                        nc.vector.tensor_add(
                                            es, fulls[0], fulls[1])
                                        fulls = []
                                        pairs.append(es)
                                        if len(pairs) == 2:
                                            eq = p2.tile(
                                                [P, QBS], BF16,
                                                name=f"eq_{b}_{h}_{qb}_{kt}",
                                                tag="eq", bufs=2)
                                            nc.vector.tensor_add(
                                                eq, pairs[0], pairs[1])
                                            pairs = []
                                            quads.append((kt + 3, eq))
                                if len(av_fifo) >= 3:
                                    emit_av(*av_fifo.pop(0))
                                av_fifo.append((kt, e_sb, lo))
                                if quads and kt >= quads[0][0]:
                                    emit_d(quads.pop(0)[1], 0, False)
                                if pend:
                                    emit_otile(*pend.pop(0))
                            # close the denominator group and start the
                            # normalization chain before draining the AV
                            # fifo, so 1/D is ready when u_ps closes
                            for _, eq in quads:
                                emit_d(eq, 0, False)
                            for dk in range(4):
                                emit_d(diag_e[dk], dk * P, dk == 3)
                            rf_sb = p2.tile([1, QBS], F32,
                                            name=f"rf_{b}_{h}_{qb}",
                                            tag="rf", bufs=2)
                            nc.vector.reciprocal_approx_fast(rf_sb, d_ps)
                            rb_sb = p2.tile([P, QBS], F32,
                                            name=f"rs_{b}_{h}_{qb}",
                                            tag="rs", bufs=2)
                            nc.gpsimd.partition_broadcast(rb_sb, rf_sb)
                            for a0 in av_fifo:
                                emit_av(*a0)
                            nc.vector.tensor_mul(
                                attn_sb[b][h][:, qb * QBS:(qb + 1) * QBS],
                                u_ps, rb_sb)

                        # queue this q-block's output tiles (heads done)
                        for i in range(4):
                            for ob2 in range(NOB // 2):
                                pend.append((b, qb * 4 + i, ob2))
                # drain whatever o-proj work is still queued
                for args in pend:
                    emit_otile(*args)

    nc.compile()
    return nc


def _prep_inputs(hidden_states, Wq, Wk, Wv, Wo, cos, sin):
    hs = np.asarray(hidden_states, dtype=np.float32)
    Wq = np.asarray(Wq, dtype=np.float32)
    Wk = np.asarray(Wk, dtype=np.float32)
    Wv = np.asarray(Wv, dtype=np.float32)
    Wo = np.asarray(Wo, dtype=np.float32)
    cos = np.asarray(cos, dtype=np.float32)
    sin = np.asarray(sin, dtype=np.float32)

    # x: [H, T] -> [NTB*P, NA*TB] so each (tb, 8-slice) load is one DMA
    # with 8KB contiguous per-partition runs
    xtm = np.ascontiguousarray(
        hs.reshape(T, H).T.reshape(NA, P, NTB, TB).transpose(2, 1, 0, 3)
        .reshape(NTB * P, NA * TB)).astype(NPBF)
    cosT = np.ascontiguousarray(cos.T).astype(NPBF)
    sinT = np.ascontiguousarray(sin.T)
    sints = np.ascontiguousarray(
        np.concatenate([-sinT[:64], sinT[64:]], axis=0)).astype(NPBF)
    kq = np.arange(P)
    trim = (kq[None, :] >= kq[:, None]).astype(NPBF)
    ident = np.eye(P, dtype=NPBF)
    onesc = np.ones((P, 1), dtype=NPBF)

    def pmajor(w):
        # [rows, cols] -> [128, (rows/128)*cols] partition-major layout
        r, ccols = w.shape
        return np.ascontiguousarray(
            w.reshape(r // P, P, ccols).transpose(1, 0, 2).reshape(P, -1)
        ).astype(NPBF)

    in_maps = []
    for c in range(8):
        in_maps.append({
            "xt": xtm,
            "wqt": pmajor(np.ascontiguousarray(Wq[c * FL:(c + 1) * FL, :].T)),
            "wkt": pmajor(np.ascontiguousarray(Wk[c * DK:(c + 1) * DK, :].T)),
            "wvt": pmajor(np.ascontiguousarray(Wv[c * DK:(c + 1) * DK, :].T)),
            "wot": pmajor(np.ascontiguousarray(Wo[:, c * FL:(c + 1) * FL].T)),
            "cost": cosT,
            "sints": sints,
            "trimask": trim,
            "identb": ident,
            "onesc": onesc,
        })
    return in_maps


def kernel(hidden_states, Wq, Wk, Wv, Wo, cos, sin, _run_kwargs=None):
    in_maps = _prep_inputs(hidden_states, Wq, Wk, Wv, Wo, cos, sin)
    if "nc" not in _NC_CACHE:
        _NC_CACHE["nc"] = build()
    nc = _NC_CACHE["nc"]
    kw = _run_kwargs or {}
    res = run_bass_kernel_spmd(nc, in_maps, core_ids=list(range(8)), **kw)
    acc = np.zeros((T, H), dtype=np.float64)
    for c in range(8):
        acc += np.asarray(res.results[c]["out"], dtype=np.float64)
    out = acc.astype(np.float32).reshape(B, S, H)
    if kw:
        _NC_CACHE["last_results"] = res
    return out


# revision 31
# speedup vs baseline: 1.1868x; 1.1868x over previous
"""Trainium2 Bass kernel for Llama GQA attention (B=2, S=2048, H=4096,
32 Q heads / 8 KV heads, head_dim 128, RoPE, causal).

Sharding: tensor-parallel by head across 8 cores. Core c owns Q heads
[4c..4c+3] and KV head c. Each core computes its Q/K/V projections,
RoPE, causal attention, and a partial output projection over its 512
attention features; the host sums the 8 partial outputs (bf16).

Layout decisions (vs a DRAM-scratch design):
  - q/k/v and attn live in SBUF end-to-end; no DRAM round trip, no
    phase-2 reload stall (which would also trigger a HAM re-throttle).
  - V is transposed to [tok, d] at eviction time with PE transposes.
  - All DRAM inputs are host-pre-transposed to partition-major layout
    so every load has large contiguous per-partition runs (small
    descriptors starve under packet-level SDMA round-robin); loads are
    issued on the ACT ring in first-needed order while the x-token
    stream owns the SP ring.
  - Phase 2 runs qb-major with qb descending (deep kt pipeline at the
    phase boundary). The softmax denominator pre-reduces full exp
    tiles 4:1 on the DVE before the ones-column matmul. o-proj tiles
    queue per finished q-block and are emitted one-per-kt inside later
    attention blocks, filling PE slack under the ACT-bound exp stream
    and spreading output DMA.
  - Output partials are written bf16 (halves output DMA bytes).
"""
import math
import numpy as np
import ml_dtypes

import concourse.bacc as bacc
import concourse.tile as tile
from concourse import mybir
from concourse.bass_utils import run_bass_kernel_spmd

F32 = mybir.dt.float32
BF16 = mybir.dt.bfloat16
NPBF = ml_dtypes.bfloat16

P = 128
B, S, H = 2, 2048, 4096
T = B * S
DK = 128
NHL = 4                      # Q heads per core
FL = NHL * DK                # 512 q features per core
TB = 512                     # token block in phase 1
NTB = T // TB
NA = H // P                  # 32 contraction slices
NAG = 4                      # a-slices per x DMA
QBS = 512                    # q block in phase 2
NQB = S // QBS
NKT = S // P
SCALE = 1.0 / math.sqrt(DK)
NOB = H // 512               # 8 output-column blocks

_NC_CACHE = {}


def build():
    nc = bacc.Bacc(None, target_bir_lowering=False)

    # weights and x arrive pre-transposed to partition-major SBUF layout so
    # each load is one DMA with large contiguous per-partition descriptors
    # (small-descriptor streams starve under packet-level SDMA round-robin)
    xt = nc.dram_tensor("xt", [NTB * P, NA * TB], BF16, kind="ExternalInput")
    wqt = nc.dram_tensor("wqt", [P, NA * FL], BF16, kind="ExternalInput")
    wkt = nc.dram_tensor("wkt", [P, NA * DK], BF16, kind="ExternalInput")
    wvt = nc.dram_tensor("wvt", [P, NA * DK], BF16, kind="ExternalInput")
    wot = nc.dram_tensor("wot", [P, NHL * H], BF16, kind="ExternalInput")
    cost = nc.dram_tensor("cost", [P, S], BF16, kind="ExternalInput")
    sints = nc.dram_tensor("sints", [P, S], BF16, kind="ExternalInput")
    trimask = nc.dram_tensor("trimask", [P, P], BF16, kind="ExternalInput")
    identb = nc.dram_tensor("identb", [P, P], BF16, kind="ExternalInput")
    onesc = nc.dram_tensor("onesc", [P, 1], BF16, kind="ExternalInput")
    out = nc.dram_tensor("out", [T, H], BF16, kind="ExternalOutput")

    EXP = mybir.ActivationFunctionType.Exp

    with nc.allow_low_precision(reason="attention compute dtypes are "
                                       "deliberately reduced"), \
         tile.TileContext(nc) as tc:
        with tc.tile_pool(name="const", bufs=1) as cp, \
             tc.tile_pool(name="resid", bufs=1) as rsp, \
             tc.tile_pool(name="wo", bufs=1) as wop:
            # ---- persistent SBUF tensors ----
            cos_sb = cp.tile([P, S], BF16)
            sin_sb = cp.tile([P, S], BF16)
            tri_sb = cp.tile([P, P], BF16)
            id_sb = cp.tile([P, P], BF16)
            oc_sb = cp.tile([P, 1], BF16)
            q_sb = [[rsp.tile([P, S], BF16, name=f"q{b}_{j}")
                     for j in range(NHL)] for b in range(B)]
            k_sb = [rsp.tile([P, S], BF16, name=f"k{b}") for b in range(B)]
            vtk_sb = [rsp.tile([P, NKT, P], BF16, name=f"vt{b}")
                      for b in range(B)]
            attn_sb = [[rsp.tile([P, S], BF16, name=f"attn{b}_{h}")
                        for h in range(NHL)] for b in range(B)]
            wo_sb = wop.tile([P, NHL, H], BF16)

            # ---- bulk loads on the ACT ring, in need-order: the first
            # token block needs wk/wv/wq[a] progressively, then the
            # eviction/RoPE chain needs id + cos/sin. wo rides the gpsimd
            # ring mid-phase-1 so it doesn't contend at startup.
            wk_sb = cp.tile([P, NA, DK], BF16)
            wv_sb = cp.tile([P, NA, DK], BF16)
            wq_sb = cp.tile([P, NA, FL], BF16)
            wk_view = wkt[:, :].rearrange("p (a f) -> p a f", a=NA)
            wv_view = wvt[:, :].rearrange("p (a f) -> p a f", a=NA)
            wq_view = wqt[:, :].rearrange("p (a f) -> p a f", a=NA)
            nc.scalar.dma_start(out=wk_sb[:, :8, :], in_=wk_view[:, :8, :])
            nc.scalar.dma_start(out=wv_sb[:, :8, :], in_=wv_view[:, :8, :])
            nc.scalar.dma_start(out=wq_sb[:, :8, :], in_=wq_view[:, :8, :])
            nc.scalar.dma_start(out=wk_sb[:, 8:, :], in_=wk_view[:, 8:, :])
            nc.scalar.dma_start(out=wv_sb[:, 8:, :], in_=wv_view[:, 8:, :])
            nc.scalar.dma_start(out=wq_sb[:, 8:16, :], in_=wq_view[:, 8:16, :])
            nc.scalar.dma_start(out=id_sb, in_=identb[:, :])
            nc.scalar.dma_start(out=oc_sb, in_=onesc[:, :])
            nc.scalar.dma_start(out=cos_sb, in_=cost[:, :])
            nc.scalar.dma_start(out=sin_sb, in_=sints[:, :])
            for g in range(2, 4):
                nc.scalar.dma_start(out=wq_sb[:, g * 8:(g + 1) * 8, :],
                                    in_=wq_view[:, g * 8:(g + 1) * 8, :])
            nc.scalar.dma_start(out=tri_sb, in_=trimask[:, :])

            # ---------------- Phase 1: QKV projection + RoPE --------------
            with tc.tile_pool(name="xp", bufs=2) as xp, \
                 tc.tile_pool(name="rp", bufs=1) as rp, \
                 tc.tile_pool(name="ps1", bufs=1, space="PSUM") as ps1:

                def rope(src, dst, s0, uid):
                    # dst = src*cos + swap_halves(src)*sints  (all bf16)
                    sw = rp.tile([P, TB], BF16, name=f"sw_{uid}",
                                 tag="sw", bufs=6)
                    nc.gpsimd.dma_start(out=sw[0:64, :], in_=src[64:128, :])
                    nc.gpsimd.dma_start(out=sw[64:128, :], in_=src[0:64, :])
                    nc.vector.tensor_mul(src, src, cos_sb[:, s0:s0 + TB])
                    nc.vector.tensor_mul(sw, sw, sin_sb[:, s0:s0 + TB])
                    nc.vector.tensor_add(dst, src, sw)

                for tb in range(NTB):
                    if tb == 4:
                        # 4MB wo load on the gpsimd ring; the tiny copy
                        # into its corner pins it behind tb3 (the Tile
                        # scheduler would otherwise hoist it to t=0 where
                        # its bulk starves the startup-critical loads)
                        nc.gpsimd.tensor_copy(wo_sb[0:1, 0, 0:2],
                                              k_sb[0][0:1, 1536:1538])
                        nc.gpsimd.dma_start(
                            out=wo_sb,
                            in_=wot[:, :].rearrange("p (j o) -> p j o",
                                                    j=NHL))
                    bi = (tb * TB) // S
                    s0 = (tb * TB) % S
                    psq = [ps1.tile([P, TB], F32, name=f"psq{j}_{tb}",
                                    tag=f"psq{j}") for j in range(NHL)]
                    psk = ps1.tile([P, TB], F32, name=f"psk_{tb}", tag="psk")
                    psv = ps1.tile([P, TB], F32, name=f"psv_{tb}", tag="psv")
                    chunks = ([(0, 4), (4, 4), (8, 8), (16, 8), (24, 8)]
                              if tb == 0 else
                              [(0, 8), (8, 8), (16, 8), (24, 8)])
                    for a0, nag in chunks:
                        x_t = xp.tile([P, 8, TB], BF16,
                                      name=f"x_{tb}_{a0}", tag="xt")
                        nc.sync.dma_start(
                            out=x_t[:, :nag, :],
                            in_=xt[tb * P:(tb + 1) * P,
                                   a0 * TB:(a0 + nag) * TB].rearrange(
                                       "p (a t) -> p a t", a=nag))
                        for ai in range(nag):
                            a = a0 + ai
                            st, sp = (a == 0), (a == NA - 1)
                            nc.tensor.matmul(psk, wk_sb[:, a, :],
                                             x_t[:, ai, :], start=st, stop=sp)
                            nc.tensor.matmul(psv, wv_sb[:, a, :],
                                             x_t[:, ai, :], start=st, stop=sp)
                            for j in range(NHL):
                                nc.tensor.matmul(
                                    psq[j],
                                    wq_sb[:, a, j * DK:(j + 1) * DK],
                                    x_t[:, ai, :], start=st, stop=sp)

                    # evictions: K first (next tb's first matmul is psk)
                    kc = rp.tile([P, TB], BF16, name=f"kc_{tb}", tag="kc",
                                 bufs=2)
                    nc.scalar.copy(kc, psk)
                    rope(kc, k_sb[bi][:, s0:s0 + TB], s0, f"k{tb}")
                    vb = rp.tile([P, TB], BF16, name=f"vb_{tb}", tag="vb",
                                 bufs=2)
                    nc.vector.tensor_copy(vb, psv)
                    vt_ps = ps1.tile([P, TB], BF16, name=f"vt_{tb}", tag="vt")
                    for m in range(4):
                        nc.tensor.transpose(vt_ps[:, m * P:(m + 1) * P],
                                            vb[:, m * P:(m + 1) * P], id_sb)
                    kt0 = (s0 // P)
                    nc.scalar.copy(vtk_sb[bi][:, kt0:kt0 + 4, :], vt_ps)
                    for j in range(NHL):
                        qc = rp.tile([P, TB], BF16, name=f"qc_{tb}_{j}",
                                     tag="qc", bufs=6)
                        if j % 2 == 0:
                            nc.vector.tensor_copy(qc, psq[j])
                        else:
                            nc.scalar.copy(qc, psq[j])
                        rope(qc, q_sb[bi][j][:, s0:s0 + TB], s0, f"q{tb}_{j}")

            # ------------- Phase 2: attention + output projection ---------
            # qb runs 3..0 so the phase starts with the deepest kt
            # pipeline (absorbs the tb7 eviction/RoPE epilogue latency).
            # Full (non-diagonal) exp tiles are pre-reduced 4:1 on the DVE
            # so the softmax-denominator matmuls stream 4x fewer columns.
            with tc.tile_pool(name="p2", bufs=1) as p2, \
                 tc.tile_pool(name="p2e", bufs=6) as p2e, \
                 tc.tile_pool(name="p3o", bufs=4) as p3o, \
                 tc.tile_pool(name="ps2s", bufs=3, space="PSUM") as ps2s, \
                 tc.tile_pool(name="ps2u", bufs=3, space="PSUM") as ps2u:
                ocnt = [0]

                def emit_otile(b, ti, ob2):
                    # one [128, 1024] tile covering output blocks 2*ob2,
                    # 2*ob2+1; a single paired DMA on the SP ring
                    o_sb = p3o.tile([P, 1024], BF16, name=f"os_{ocnt[0]}",
                                    tag="os")
                    for half in range(2):
                        ob = 2 * ob2 + half
                        o_ps = ps2u.tile([P, 512], F32,
                                         name=f"o_{ocnt[0]}_{half}",
                                         tag="u", bufs=3)
                        for j in range(NHL):
                            nc.tensor.matmul(
                                o_ps, attn_sb[b][j][:, ti * P:(ti + 1) * P],
                                wo_sb[:, j, ob * 512:(ob + 1) * 512],
                                start=(j == 0), stop=(j == NHL - 1))
                        dst = o_sb[:, half * 512:(half + 1) * 512]
                        # evicts stay off ACT: the exp stream is the
                        # attention-phase bottleneck and DVE has slack
                        nc.vector.tensor_copy(dst, o_ps)
                    r0 = b * S + ti * P
                    eng = nc.sync if ocnt[0] % 2 == 0 else nc.scalar
                    eng.dma_start(
                        out=out[r0:r0 + P, ob2 * 1024:(ob2 + 1) * 1024],
                        in_=o_sb)
                    ocnt[0] += 1

                # o-proj tiles are produced per finished q-block and
                # consumed one-per-kt inside later attention blocks, so
                # PE fills the slack while ACT streams the exps
                pend = []
                for b in range(B):
                    for qb in range(NQB - 1, -1, -1):
                        nkt = 4 * qb + 4
                        for h in range(NHL):
                            u_ps = ps2u.tile([P, QBS], F32,
                                             name=f"u_{b}_{h}_{qb}", tag="u",
                                             bufs=3)
                            d_ps = ps2u.tile([1, QBS], F32,
                                             name=f"d_{b}_{h}_{qb}", tag="d",
                                             bufs=2)
                            dflag = [True]

                            def emit_d(src, lo, sp, d_ps=d_ps):
                                nc.tensor.matmul(d_ps[:, lo:], oc_sb,
                                                 src[:, lo:],
                                                 start=dflag[0], stop=sp,
                                                 skip_group_check=True)
                                dflag[0] = False

                            def emit_av(kt, e_sb, lo, u_ps=u_ps, nkt=nkt,
                                        b=b):
                                st, sp = (kt == 0), (kt == nkt - 1)
                                nc.tensor.matmul(u_ps[:, lo:],
                                                 vtk_sb[b][:, kt, :],
                                                 e_sb[:, lo:],
                                                 start=st, stop=sp,
                                                 skip_group_check=True)

                            av_fifo = []
                            diag_e = []   # diagonal e tiles (post-mask)
                            fulls = []    # full e tiles awaiting pair add
                            pairs = []    # pair sums awaiting quad add
                            quads = []    # (ready_kt, quad tile) for d-mm
                            for kt in range(nkt):
                                s_ps = ps2s.tile(
                                    [P, QBS], F32,
                                    name=f"s_{b}_{h}_{qb}_{kt}", tag="s")
                                m = kt - 4 * qb
                                lo = m * P if m > 0 else 0
                                nc.tensor.matmul(
                                    s_ps[:, lo:],
                                    k_sb[b][:, kt * P:(kt + 1) * P],
                                    q_sb[b][h][:, qb * QBS + lo:
                                               (qb + 1) * QBS],
                                    start=True, stop=True)
                                e_sb = p2e.tile(
                                    [P, QBS], BF16,
                                    name=f"e_{b}_{h}_{qb}_{kt}", tag="e")
                                nc.scalar.activation(e_sb[:, lo:],
                                                     s_ps[:, lo:], EXP,
                                                     scale=SCALE)
                                if m >= 0:
                                    nc.vector.tensor_mul(
                                        e_sb[:, m * P:(m + 1) * P],
                                        e_sb[:, m * P:(m + 1) * P],
                                        tri_sb)
                                    diag_e.append(e_sb)
                                else:
                                    # 4:1 DVE pre-reduction of full tiles
                                    # for the denominator matmul
                                    fulls.append(e_sb)
                                    if len(fulls) == 2:
                                        es = p2.tile(
                                            [P, QBS], BF16,
                                            name=f"ep_{b}_{h}_{qb}_{kt}",
                                            tag="ep", bufs=3)
                                        nc.vector.tensor_add(
                                            es, fulls[0], fulls[1])
                                        fulls = []
                                        pairs.append(es)
                                        if len(pairs) == 2:
                                            eq = p2.tile(
                                                [P, QBS], BF16,
                                                name=f"eq_{b}_{h}_{qb}_{kt}",
                                                tag="eq", bufs=2)
                                            nc.vector.tensor_add(
                                                eq, pairs[0], pairs[1])
                                            pairs = []
                                            quads.append((kt + 3, eq))
                                if len(av_fifo) >= 3:
                                    emit_av(*av_fifo.pop(0))
                                av_fifo.append((kt, e_sb, lo))
                                if quads and kt >= quads[0][0]:
                                    emit_d(quads.pop(0)[1], 0, False)
                                if pend:
                                    emit_otile(*pend.pop(0))
                            # close the denominator group and start the
                            # normalization chain before draining the AV
                            # fifo, so 1/D is ready when u_ps closes
                            for _, eq in quads:
                                emit_d(eq, 0, False)
                            for dk in range(4):
                                emit_d(diag_e[dk], dk * P, dk == 3)
                            rf_sb = p2.tile([1, QBS], F32,
                                            name=f"rf_{b}_{h}_{qb}",
                                            tag="rf", bufs=2)
                            nc.vector.reciprocal_approx_fast(rf_sb, d_ps)
                            rb_sb = p2.tile([P, QBS], F32,
                                            name=f"rs_{b}_{h}_{qb}",
                                            tag="rs", bufs=2)
                            nc.gpsimd.partition_broadcast(rb_sb, rf_sb)
                            for a0 in av_fifo:
                                emit_av(*a0)
                            nc.vector.tensor_mul(
                                attn_sb[b][h][:, qb * QBS:(qb + 1) * QBS],
                                u_ps, rb_sb)

                        # queue this q-block's output tiles (heads done)
                        for i in range(4):
                            for ob2 in range(NOB // 2):
                                pend.append((b, qb * 4 + i, ob2))
                # drain whatever o-proj work is still queued
                for args in pend:
                    emit_otile(*args)

    nc.compile()
    return nc


def _prep_inputs(hidden_states, Wq, Wk, Wv, Wo, cos, sin):
    hs = np.asarray(hidden_states, dtype=np.float32)
    Wq = np.asarray(Wq, dtype=np.float32)
    Wk = np.asarray(Wk, dtype=np.float32)
    Wv = np.asarray(Wv, dtype=np.float32)
    Wo = np.asarray(Wo, dtype=np.float32)
    cos = np.asarray(cos, dtype=np.float32)
    sin = np.asarray(sin, dtype=np.float32)

    # x: [H, T] -> [NTB*P, NA*TB] so each (tb, 8-slice) load is one DMA
    # with 8KB contiguous per-partition runs
    xtm = np.ascontiguousarray(
        hs.reshape(T, H).T.reshape(NA, P, NTB, TB).transpose(2, 1, 0, 3)
        .reshape(NTB * P, NA * TB)).astype(NPBF)
    cosT = np.ascontiguousarray(cos.T).astype(NPBF)
    sinT = np.ascontiguousarray(sin.T)
    sints = np.ascontiguousarray(
        np.concatenate([-sinT[:64], sinT[64:]], axis=0)).astype(NPBF)
    kq = np.arange(P)
    trim = (kq[None, :] >= kq[:, None]).astype(NPBF)
    ident = np.eye(P, dtype=NPBF)
    onesc = np.ones((P, 1), dtype=NPBF)

    def pmajor(w):
        # [rows, cols] -> [128, (rows/128)*cols] partition-major layout
        r, ccols = w.shape
        return np.ascontiguousarray(
            w.reshape(r // P, P, ccols).transpose(1, 0, 2).reshape(P, -1)
        ).astype(NPBF)

    in_maps = []
    for c in range(8):
        in_maps.append({
            "xt": xtm,
            "wqt": pmajor(np.ascontiguousarray(Wq[c * FL:(c + 1) * FL, :].T)),
            "wkt": pmajor(np.ascontiguousarray(Wk[c * DK:(c + 1) * DK, :].T)),
            "wvt": pmajor(np.ascontiguousarray(Wv[c * DK:(c + 1) * DK, :].T)),
            "wot": pmajor(np.ascontiguousarray(Wo[:, c * FL:(c + 1) * FL].T)),
            "cost": cosT,
            "sints": sints,
            "trimask": trim,
            "identb": ident,
            "onesc": onesc,
        })
    return in_maps


def kernel(hidden_states, Wq, Wk, Wv, Wo, cos, sin, _run_kwargs=None):
    in_maps = _prep_inputs(hidden_states, Wq, Wk, Wv, Wo, cos, sin)
    if "nc" not in _NC_CACHE:
        _NC_CACHE["nc"] = build()
    nc = _NC_CACHE["nc"]
    kw = _run_kwargs or {}
    res = run_bass_kernel_spmd(nc, in_maps, core_ids=list(range(8)), **kw)
    acc = np.zeros((T, H), dtype=np.float64)
    for c in range(8):
        acc += np.asarray(res.results[c]["out"], dtype=np.float64)
    out = acc.astype(np.float32).reshape(B, S, H)
    if kw:
        _NC_CACHE["last_results"] = res
    return out
